# revision 6
# baseline (speedup 1.0000x reference)
"""Constrained sparsemax (topk_masking) Trainium2 Bass kernel — v4.

probs[r] = clip(z[r] - tau_r, 0, u[r]) with per-row tau_r s.t. row sums to 1.

Device algorithm per 128-row tile (4 tiles per core, 8 cores), fully
pipelined per tile (load -> scan -> top-k -> gather overlap across tiles):

  1. One 2 MiB DMA load of the tile's z in bf16 (halves the dense HBM read).
  2. Per-row max over 256 buckets of 32 via a pairwise bf16 max tree on the
     DVE (tensor_tensor runs at 2x the rate of tensor_reduce).
  3. Bit-jitter bucket maxima (bucket idx in low 8 mantissa bits) so top-k
     selection is tie-free;
     top-13 buckets via max8 + match_replace.  Bucket 13's max `bound` is
     (empirically) a lower bound for tau*.
  4. ONE indirect-DMA per tile gathers the top-12 blocks from a host-built
     table whose 256B rows are each 32-block's values sorted descending by z
     with their matching u's ([z32|u32]; rows below 256B return garbage on
     HW).  A single instruction per tile keeps the serial SWDGE
     descriptor-generation cost on GpSimd at ~1.5us/tile (chunking pays
     ~1us fixed cost per chunk).
  5. The device compacts only the top TK=8 of each gathered block (an
     empirically lossless truncation, certified per row via the flg check)
     and runs the batched tau iteration for all 4 tiles on the DVE only
     (no cross-engine sync): 2 bisection + 3 guarded, bracket-clamped
     secant steps over the 96-wide compacted f32 data, with per-stream
     reductions in [P,4] accumulator slots.
  6. Device emits exact probabilities for the gathered values
     (pc = clip(zc-tau, 0, uc)), block ids (blk), the final residual f(tau)
     (ff), and a bound-margin flag (flg).  flg covers both the 13th-bucket
     bound AND the per-block TK-th-largest value (the eval truncates blocks
     to their top TK).

The dense output is NOT written by the device: every coordinate outside the
evaluated top-TK-of-top-12-blocks provably satisfies z <= bound <= tau
(checked per row via flg), so its probability is exactly 0.  The host
materializes zeros + scatters pc; rows with flg > 0 or |ff - 1| > 5e-3
(tau iteration misconverged; final probs would still be ~1e-3 accurate) or
failing a consistency check are recomputed exactly on the host
(~150 of 4096 rows; target accuracy is rel err < 2e-2, kept-row error is
~4e-3 worst case).

Sharding: batch rows split evenly across 8 NeuronCores (data parallel).
"""

import sys

for _p in ("/opt/trn_rl_repo", "/opt/pypackages"):
    if _p not in sys.path:
        sys.path.append(_p)

import numpy as np
import ml_dtypes

import concourse.bass as bass
import concourse.bacc as bacc
import concourse.tile as tile
import concourse.mybir as mybir
from concourse.bass_utils import run_bass_kernel_spmd

F32 = mybir.dt.float32
BF16 = mybir.dt.bfloat16
U32 = mybir.dt.uint32
I32 = mybir.dt.int32
Alu = mybir.AluOpType
AxX = mybir.AxisListType.X

B, N = 4096, 8192
NCORES = 8
ROWS = B // NCORES          # 512 rows per core
P = 128                     # partitions
NT = ROWS // P              # 4 tiles per core
NB, BSZ = 256, 32           # buckets per row / bucket size
T = 12                      # buckets gathered per row
TK = 8                      # values kept per gathered bucket (host-sorted)
CW = T * TK                 # compacted row width (192)
K_BISECT = 2
S_SECANT = 3
MARGIN = 0.02               # flag rows where bound is this close to tau
RESID_TOL = 5e-3            # host-side |f(tau)-1| misconvergence tolerance

NEG_INF = -1.0e30  # effectively -inf; literal inf breaks BIR JSON serialization

NP_BF16 = np.dtype(ml_dtypes.bfloat16)


def _emit(nc: bass.Bass) -> None:
    zb_d = nc.dram_tensor("zb", [ROWS, N], BF16, kind="ExternalInput")
    zu_d = nc.dram_tensor("zu", [ROWS * NB, 2 * BSZ], F32, kind="ExternalInput")
    iota_d = nc.dram_tensor("iota", [P, NB], U32, kind="ExternalInput")
    rowb_d = nc.dram_tensor("rowb", [P, NT], U32, kind="ExternalInput")
    pc_d = nc.dram_tensor("pc", [ROWS, CW], F32, kind="ExternalOutput")
    blk_d = nc.dram_tensor("blk", [ROWS, T], I32, kind="ExternalOutput")
    flg_d = nc.dram_tensor("flg", [P, NT], F32, kind="ExternalOutput")
    ff_d = nc.dram_tensor("ff", [P, NT], F32, kind="ExternalOutput")

    zu_blocks = zu_d.ap()

    with tile.TileContext(nc) as tc:
        with (
            tc.tile_pool(name="big", bufs=4) as bigp,       # bf16 z tiles
            tc.tile_pool(name="cw", bufs=1) as cwp,         # compacted tensors
            tc.tile_pool(name="scr", bufs=1) as scrp,       # eval scratch
            tc.tile_pool(name="sml", bufs=2) as smlp,       # bucket-sized tensors
            tc.tile_pool(name="tiny", bufs=3) as tinyp,     # [P,4] scalars
            tc.tile_pool(name="const", bufs=1) as cstp,
        ):
            iot = cstp.tile([P, NB], U32, tag="iota")
            rwb = cstp.tile([P, NT], U32, tag="rowb")
            zeros = cstp.tile([P, CW], F32, tag="zeros")
            ones4 = cstp.tile([P, NT], F32, tag="ones4")
            nc.sync.dma_start(out=iot[:], in_=iota_d.ap())
            nc.sync.dma_start(out=rwb[:], in_=rowb_d.ap())
            nc.vector.memset(zeros[:], 0.0)
            nc.vector.memset(ones4[:], 1.0)

            bound4 = cstp.tile([P, NT], F32, tag="bound4")
            m14 = cstp.tile([P, NT], F32, tag="m14")
            t16c = cstp.tile([P, NT], F32, tag="t16c")

            # all 4 tile loads issued up-front on one HWDGE queue: they drain
            # in order, so tile t's data lands ~t*4.9us and the DVE tree
            # pipelines behind the loads
            zts = []
            for t in range(NT):
                r0 = t * P
                zt = bigp.tile([P, N], BF16, tag="zt", name=f"zt{t}")
                nc.sync.dma_start(out=zt[:], in_=zb_d.ap()[r0:r0 + P, :])
                zts.append(zt)

            zcu, zcc, wcc = {}, {}, {}
            scr_z, scr_w = {}, {}
            for s in range(NT):
                zcu[s] = cwp.tile([P, T, 2 * BSZ], F32, tag=f"zcu{s}",
                                  name=f"zcu{s}")
                zcc[s] = cwp.tile([P, T, TK], F32, tag=f"zcc{s}",
                                  name=f"zcc{s}")
                wcc[s] = cwp.tile([P, T, TK], F32, tag=f"wcc{s}",
                                  name=f"wcc{s}")
                scr_z[s] = scrp.tile([P, CW], F32, tag=f"scr_z{s}",
                                     name=f"scr_z{s}")
                scr_w[s] = scrp.tile([P, CW], F32, tag=f"scr_w{s}",
                                     name=f"scr_w{s}")

            def uview(s):
                return zcu[s][:, :, BSZ:BSZ + TK]

            def flat(tl):
                return tl[:].rearrange("p t s -> p (t s)")

            def front(t):
                """scan + top-k + gather for one tile."""
                r0 = t * P
                zt = zts[t]

                # --- bucket max: pairwise bf16 max rounds on the DVE --------
                cur = zt[:].rearrange("p (nb s) -> p nb s", nb=NB)
                w = BSZ
                while w > 2:
                    nxt = smlp.tile([P, NB, w // 2], BF16, tag=f"pm{w}",
                                    name=f"pm{w}_{t}")
                    nc.vector.tensor_tensor(
                        nxt[:], cur[:, :, 0:w // 2], cur[:, :, w // 2:w],
                        Alu.max)
                    cur = nxt[:]
                    w //= 2
                bm = smlp.tile([P, NB], F32, tag="bm", name=f"bm_{t}")
                nc.vector.tensor_tensor(
                    bm[:].rearrange("p (nb s) -> p nb s", nb=NB),
                    cur[:, :, 0:1], cur[:, :, 1:2], Alu.max)

                # --- bit-jitter (GpSimd): bucket idx into low 8 mantissa bits
                bmj = smlp.tile([P, NB], F32, tag="bmj", name=f"bmj_{t}")
                nc.vector.tensor_tensor(
                    bmj[:].bitcast(U32), bm[:].bitcast(U32), iot[:],
                    Alu.bitwise_or)

                # --- top-13 buckets (12 gathered + 13th as bound) -----------
                m16 = smlp.tile([P, 16], F32, tag="m16", name=f"m16_{t}")
                nc.vector.max(m16[:, 0:8], bmj[:])
                bmr = smlp.tile([P, NB], F32, tag="bmr", name=f"bmr_{t}")
                nc.vector.match_replace(bmr[:], m16[:, 0:8], bmj[:], NEG_INF)
                nc.vector.max(m16[:, 8:16], bmr[:])
                nc.vector.tensor_copy(bound4[:, t:t + 1], m16[:, T:T + 1])
                nc.vector.tensor_copy(m14[:, t:t + 1], m16[:, 0:1])

                # --- gather indices -----------------------------------------
                sel = smlp.tile([P, T], U32, tag="sel", name=f"sel{t}")
                nc.vector.tensor_scalar(
                    sel[:], m16[:, 0:T].bitcast(U32), 0xFF, None,
                    Alu.bitwise_and)
                blk = smlp.tile([P, T], I32, tag=f"blk{t}", name=f"blk{t}")
                nc.vector.tensor_tensor(
                    blk[:].bitcast(U32), sel[:],
                    rwb[:, t:t + 1].broadcast_to((P, T)), Alu.add)
                nc.sync.dma_start(out=blk_d.ap()[r0:r0 + P, :], in_=blk[:])

                # --- ONE indirect gather for the whole tile -----------------
                nc.gpsimd.indirect_dma_start(
                    out=zcu[t][:, :, :], out_offset=None, in_=zu_blocks,
                    in_offset=bass.IndirectOffsetOnAxis(ap=blk[:], axis=0))

                # --- compact z and w = z - u (contiguous; accumulating stt
                # on HW needs contiguous operands) + per-block last-value bound
                nc.vector.tensor_copy(zcc[t][:], zcu[t][:, :, 0:TK])
                nc.vector.tensor_tensor(
                    wcc[t][:], zcu[t][:, :, 0:TK], uview(t), Alu.subtract)
                tlast = smlp.tile([P, T], F32, tag="tlast", name=f"tlast{t}")
                nc.vector.tensor_copy(
                    tlast[:], zcc[t][:, :, TK - 1:TK].rearrange(
                        "p a b -> p (a b)"))
                nc.vector.tensor_reduce(t16c[:, t:t + 1], tlast[:], AxX,
                                        Alu.max)

            for t in range(NT):
                front(t)

            # --- batched tau iteration over all 4 streams (DVE only) --------
            hh = tinyp.tile([P, NT], F32, tag="hh")
            nc.vector.tensor_tensor(hh[:], m14[:], bound4[:], Alu.subtract)
            h4 = tinyp.tile([P, NT], F32, tag="h4")
            nc.vector.tensor_scalar(h4[:], hh[:], 0.5, None, Alu.mult)
            lo4 = tinyp.tile([P, NT], F32, tag="lo4")
            nc.vector.tensor_copy(lo4[:], bound4[:])
            tau4 = tinyp.tile([P, NT], F32, tag="tau4")
            nc.vector.tensor_tensor(tau4[:], lo4[:], h4[:], Alu.add)
            ntau4 = tinyp.tile([P, NT], F32, tag="ntau4")
            nc.vector.tensor_scalar(ntau4[:], tau4[:], -1.0, None, Alu.mult)

            def eval_f():
                """f(tau4) per stream -> f4 [P,4] (rz - rw), all on DVE."""
                rz4 = tinyp.tile([P, NT], F32, tag="rz4")
                rw4 = tinyp.tile([P, NT], F32, tag="rw4")
                for s in range(NT):
                    nc.vector.scalar_tensor_tensor(
                        scr_z[s][:], flat(zcc[s]), ntau4[:, s:s + 1], zeros[:],
                        Alu.add, Alu.max, accum_out=rz4[:, s:s + 1])
                for s in range(NT):
                    nc.vector.scalar_tensor_tensor(
                        scr_w[s][:], flat(wcc[s]), ntau4[:, s:s + 1], zeros[:],
                        Alu.add, Alu.max, accum_out=rw4[:, s:s + 1])
                f4 = tinyp.tile([P, NT], F32, tag="f4")
                nc.vector.tensor_tensor(f4[:], rz4[:], rw4[:], Alu.subtract)
                return f4

            tp4 = None  # previous (tau, f) for secant
            fp4 = None
            for k in range(K_BISECT):
                f4 = eval_f()
                tp4, fp4 = tau4, f4
                mask4 = tinyp.tile([P, NT], F32, tag="mask4")
                nc.vector.tensor_scalar(mask4[:], f4[:], 1.0, None, Alu.is_gt)
                mh4 = tinyp.tile([P, NT], F32, tag="mh4")
                nc.vector.tensor_tensor(mh4[:], mask4[:], h4[:], Alu.mult)
                lo4n = tinyp.tile([P, NT], F32, tag="lo4")
                nc.vector.tensor_tensor(lo4n[:], lo4[:], mh4[:], Alu.add)
                lo4 = lo4n
                h4n = tinyp.tile([P, NT], F32, tag="h4")
                nc.vector.tensor_scalar(h4n[:], h4[:], 0.5, None, Alu.mult)
                h4 = h4n
                tau4 = tinyp.tile([P, NT], F32, tag="tau4")
                nc.vector.tensor_tensor(tau4[:], lo4[:], h4[:], Alu.add)
                ntau4 = tinyp.tile([P, NT], F32, tag="ntau4")
                nc.vector.tensor_scalar(ntau4[:], tau4[:], -1.0, None, Alu.mult)

            for si in range(S_SECANT):
                f4 = eval_f()
                dn4 = tinyp.tile([P, NT], F32, tag="dn4")
                nc.vector.tensor_tensor(dn4[:], f4[:], fp4[:], Alu.subtract)
                ad4 = tinyp.tile([P, NT], F32, tag="ad4")
                nc.vector.tensor_scalar(
                    ad4[:].bitcast(U32), dn4[:].bitcast(U32), 0x7FFFFFFF, None,
                    Alu.bitwise_and)
                ok4 = tinyp.tile([P, NT], F32, tag="ok4")
                nc.vector.tensor_scalar(ok4[:], ad4[:], 1e-7, None, Alu.is_gt)
                # den = dn + (ok - 1): equals dn when ok, dn-1 (~ -1) when not
                den4 = tinyp.tile([P, NT], F32, tag="den4")
                nc.vector.scalar_tensor_tensor(
                    den4[:], ok4[:], -1.0, dn4[:], Alu.add, Alu.add)
                rec4 = tinyp.tile([P, NT], F32, tag="rec4")
                nc.vector.reciprocal(rec4[:], den4[:])
                nf4 = tinyp.tile([P, NT], F32, tag="nf4")
                nc.vector.scalar_tensor_tensor(
                    nf4[:], f4[:], -1.0, ones4[:], Alu.mult, Alu.add)
                dt4 = tinyp.tile([P, NT], F32, tag="dt4")
                nc.vector.tensor_tensor(dt4[:], tau4[:], tp4[:], Alu.subtract)
                s14 = tinyp.tile([P, NT], F32, tag="s14")
                nc.vector.tensor_tensor(s14[:], nf4[:], dt4[:], Alu.mult)
                s24 = tinyp.tile([P, NT], F32, tag="s24")
                nc.vector.tensor_tensor(s24[:], s14[:], rec4[:], Alu.mult)
                s34 = tinyp.tile([P, NT], F32, tag="s34")
                nc.vector.tensor_tensor(s34[:], s24[:], ok4[:], Alu.mult)
                tp4, fp4 = tau4, f4
                tr4 = tinyp.tile([P, NT], F32, tag="tr4")
                nc.vector.tensor_tensor(tr4[:], tp4[:], s34[:], Alu.add)
                # clamp the step to the bracket [lo4, m14]
                tc4 = tinyp.tile([P, NT], F32, tag="tc4")
                nc.vector.tensor_tensor(tc4[:], tr4[:], lo4[:], Alu.max)
                tau4 = tinyp.tile([P, NT], F32, tag="tau4")
                nc.vector.tensor_tensor(tau4[:], tc4[:], m14[:], Alu.min)
                ntau4 = tinyp.tile([P, NT], F32, tag="ntau4")
                nc.vector.tensor_scalar(ntau4[:], tau4[:], -1.0, None, Alu.mult)

            # --- outputs (pc first so its stores overlap the residual eval) --
            bnd2 = tinyp.tile([P, NT], F32, tag="bnd2")
            nc.vector.tensor_tensor(bnd2[:], bound4[:], t16c[:], Alu.max)
            flg4 = tinyp.tile([P, NT], F32, tag="flg4")
            nc.vector.scalar_tensor_tensor(
                flg4[:], bnd2[:], MARGIN, tau4[:], Alu.add, Alu.subtract)
            nc.sync.dma_start(out=flg_d.ap(), in_=flg4[:])

            for t in range(NT):
                r0 = t * P
                pc1 = cwp.tile([P, T, TK], F32, tag=f"pc1_{t}")
                nc.vector.scalar_tensor_tensor(
                    pc1[:], zcc[t][:], ntau4[:, t:t + 1], uview(t),
                    Alu.add, Alu.min)
                pcf = cwp.tile([P, CW], F32, tag=f"pcf_{t}")
                nc.vector.tensor_scalar(
                    pcf[:], flat(pc1), 0.0, None, Alu.max)
                nc.sync.dma_start(out=pc_d.ap()[r0:r0 + P, :], in_=pcf[:])

            # final residual (exported raw; host checks |ff-1| > RESID_TOL)
            ffin4 = eval_f()
            nc.sync.dma_start(out=ff_d.ap(), in_=ffin4[:])


_CACHE: dict = {}


def _get_nc() -> bass.Bass:
    if "nc" not in _CACHE:
        nc = bacc.Bacc("TRN2", target_bir_lowering=False, debug=False)
        _emit(nc)
        nc.compile()
        _CACHE["nc"] = nc
    return _CACHE["nc"]


def _const_inputs() -> dict:
    return {
        "iota": np.arange(NB, dtype=np.uint32)[None, :].repeat(P, 0).copy(),
        "rowb": ((np.arange(NT, dtype=np.uint32)[None, :] * P
                  + np.arange(P, dtype=np.uint32)[:, None]) * NB).copy(),
    }


def _make_zu(z: np.ndarray, u: np.ndarray) -> tuple[np.ndarray, np.ndarray]:
    """Per block of 32: z values sorted descending + matching u (256B rows;
    the indirect gather needs >= 256B-aligned rows on HW).  The device only
    reads/evals the top TK of each block.

    Returns (zu [B*NB, 64] f32, cols [B*NB, TK] int8) where cols are the
    within-block column indices of the top-TK sorted values."""
    zr = z.reshape(-1, BSZ)
    ur = u.reshape(-1, BSZ)
    ordr = np.argsort(-zr, 1, kind="stable")
    zu = np.empty((zr.shape[0], 2 * BSZ), dtype=np.float32)
    zu[:, :BSZ] = np.take_along_axis(zr, ordr, 1)
    zu[:, BSZ:] = np.take_along_axis(ur, ordr, 1)
    return zu, ordr[:, :TK].astype(np.int8)


def _pack_bf16(z: np.ndarray) -> np.ndarray:
    """Truncate f32 -> bf16 (round toward zero keeps z' <= |z| monotonic)."""
    return (z.view(np.uint32) >> 16).astype(np.uint16).view(NP_BF16)


def _exact_rows(z: np.ndarray, u: np.ndarray) -> np.ndarray:
    """Reference-style exact solve for a handful of rows (f64 bisection)."""
    z = z.astype(np.float64)
    u = u.astype(np.float64)
    lo = (z - u).min(1, keepdims=True)
    hi = z.max(1, keepdims=True)
    for _ in range(60):
        mid = 0.5 * (lo + hi)
        f = np.clip(z - mid, 0, u).sum(1, keepdims=True)
        big = f > 1.0
        lo = np.where(big, mid, lo)
        hi = np.where(big, hi, mid)
    tau = 0.5 * (lo + hi)
    d = z - tau
    r1 = (d > 0) & (d < u)
    r2 = d >= u
    nA = r1.sum(1, keepdims=True)
    tau2 = ((r1 * z).sum(1, keepdims=True) + (r2 * u).sum(1, keepdims=True)
            - 1.0) / np.maximum(nA, 1)
    tau = np.where(nA > 0, tau2, tau)
    return (r1 * (z - tau) + r2 * u).astype(np.float32)


def _assemble_core(out_rows: np.ndarray, pc: np.ndarray, blk: np.ndarray,
                   flg: np.ndarray, ff: np.ndarray,
                   z_rows: np.ndarray, u_rows: np.ndarray,
                   cols_rows: np.ndarray) -> None:
    """Fill one core's [ROWS, N] output: scatter exact values, then exact
    host recompute for flagged / misconverged / inconsistent rows.

    Consistency net: the device gather has a rare (deterministic,
    partition-0) erratum where a block's data is fetched from a stale
    offset.  Host-side we know blk and the true z/u, so we verify that pc
    matches clip(zc - tau, 0, uc) for a single tau; rows failing the check
    are recomputed exactly."""
    nr = out_rows.shape[0]
    bi = blk.ravel()                              # [nr*T] block row ids
    cc = cols_rows[bi].astype(np.intp)            # [nr*T, TK] column ids
    flat = out_rows.reshape(-1, BSZ)
    flat[bi[:, None], cc] = pc.reshape(-1, TK)
    zc = z_rows.reshape(-1, BSZ)[bi[:, None], cc].reshape(nr, T, TK)
    uc = u_rows.reshape(-1, BSZ)[bi[:, None], cc].reshape(nr, T, TK)
    pcb = pc.reshape(nr, T, TK)
    free = (pcb > 1e-7) & (pcb < uc - 1e-7)
    tau_est = np.where(free, zc - pcb, -np.inf).max((1, 2))
    has_free = np.isfinite(tau_est)
    pc_chk = np.clip(zc - tau_est[:, None, None], 0.0, uc)
    mism = np.abs(pc_chk - pcb).max((1, 2))
    bad = np.flatnonzero((flg.T.ravel() > 0)
                         | (np.abs(ff.T.ravel() - 1.0) > RESID_TOL)
                         | ~np.isfinite(ff.T.ravel())
                         | ~has_free
                         | (mism > 1e-4))
    if bad.size:
        out_rows[bad] = _exact_rows(z_rows[bad], u_rows[bad])


def kernel(input1: np.ndarray, input2: np.ndarray, **_ignored) -> np.ndarray:
    z = np.ascontiguousarray(np.asarray(input1, dtype=np.float32))
    u = np.ascontiguousarray(np.asarray(input2, dtype=np.float32))
    assert z.shape == (B, N) and u.shape == (B, N)
    nc = _get_nc()
    consts = _const_inputs()
    zu_all, cols_all = _make_zu(z, u)
    zu_all = zu_all.reshape(NCORES, ROWS * NB, 2 * BSZ)
    cols_all = cols_all.reshape(NCORES, ROWS * NB, TK)
    in_maps = []
    for c in range(NCORES):
        zs = z[c * ROWS:(c + 1) * ROWS]
        in_maps.append({"zb": _pack_bf16(zs), "zu": zu_all[c], **consts})
    res = run_bass_kernel_spmd(
        nc, in_maps, list(range(NCORES)), **_CACHE.get("run_kwargs", {}))
    _CACHE["last_results"] = res
    out = np.zeros((B, N), dtype=np.float32)
    for c in range(NCORES):
        r = res.results[c]
        _assemble_core(out[c * ROWS:(c + 1) * ROWS], r["pc"], r["blk"],
                       r["flg"], r["ff"], z[c * ROWS:(c + 1) * ROWS],
                       u[c * ROWS:(c + 1) * ROWS], cols_all[c])
    return out
